# revision 3
# baseline (speedup 1.0000x reference)
"""Trainium2 Bass kernel for the 3-layer tanh-RNN scan (nn_FCN_81913616269757).

Math (per step t over T=524288):
    h0 = tanh(W_ih0 x_t + b_ih0 + W_hh0 h0 + b_hh0)
    h1 = tanh(W_ih1 h0  + b_ih1 + W_hh1 h1 + b_hh1)
    hp = tanh(Wp_ih h1  + bp_ih + Wp_hh hp + bp_hh)   -> output trace hp [T, 1]

Strategy: the recurrence is strongly contractive (spectral radius ~0.6),
so the sequence is split into many independent chunks, each re-deriving
its initial state with a tau-step warmup from zeros (error decays below
the f32r noise floor within ~16 steps). Chunks are batched as columns of
a [128, B] state tile (4 block-diagonal weight groups x B columns per
core), turning each step into two accumulating f32r matmuls + one tanh
activation per layer. All 8 cores run disjoint chunk sets (data
parallel, no collectives).
"""
import numpy as np

# ---- hardcoded problem geometry ----
T = 524288
H = 32
NCORES = 8
G = 4            # groups (block-diagonal packing of the 128 partitions)
B = 256          # chunk columns per group
TAU = 32         # warmup steps per chunk
CH = G * B       # chunks per core = 1024
L = T // (NCORES * CH)   # positions per chunk = 64
S = TAU + L      # steps per pass = 96
SS = 16          # steps per x-in / y-out DMA super-slot

_cache = {}


def _round_f32r(a):
    """Round fp32 ndarray to f32r grid (11 mantissa bits, RTN)."""
    a = np.ascontiguousarray(a, np.float32)
    b = a.view(np.uint32).astype(np.uint64)
    b = (b + 0x800) & 0xFFFFF000
    return (b & 0xFFFFFFFF).astype(np.uint32).view(np.float32)


def _blockdiag4(w):
    """[32,32] -> [128,128] block diagonal, 4 copies."""
    out = np.zeros((128, 128), np.float32)
    for g in range(G):
        out[32 * g:32 * g + 32, 32 * g:32 * g + 32] = w
    return out


def _build_program():
    import concourse.bacc as bacc
    import concourse.mybir as mybir
    from concourse.tile import TileContext

    f32 = mybir.dt.float32
    f32r = mybir.dt.float32r
    Tanh = mybir.ActivationFunctionType.Tanh

    nc = bacc.Bacc("TRN2", target_bir_lowering=False, debug=False)
    xs_d = nc.dram_tensor("xs", [G, S * B], f32r, kind="ExternalInput")
    h0i_d = nc.dram_tensor("h0i", [128, B], f32r, kind="ExternalInput")
    h1i_d = nc.dram_tensor("h1i", [128, B], f32r, kind="ExternalInput")
    hpi_d = nc.dram_tensor("hpi", [G, B], f32r, kind="ExternalInput")
    bdw0_d = nc.dram_tensor("bdw0", [128, 128], f32r, kind="ExternalInput")
    wx_d = nc.dram_tensor("wx", [G, 128], f32r, kind="ExternalInput")
    bdw1i_d = nc.dram_tensor("bdw1i", [128, 128], f32r, kind="ExternalInput")
    bdw1h_d = nc.dram_tensor("bdw1h", [128, 128], f32r, kind="ExternalInput")
    wp1_d = nc.dram_tensor("wp1", [128, G], f32r, kind="ExternalInput")
    wph_d = nc.dram_tensor("wph", [G, G], f32r, kind="ExternalInput")
    b0_d = nc.dram_tensor("b0", [128, 1], f32, kind="ExternalInput")
    b1_d = nc.dram_tensor("b1", [128, 1], f32, kind="ExternalInput")
    bp_d = nc.dram_tensor("bp", [G, 1], f32, kind="ExternalInput")
    ys_d = nc.dram_tensor("ys", [G, S * B], f32r, kind="ExternalOutput")

    with TileContext(nc) as tc:
        with tc.tile_pool(name="w", bufs=1) as wp, \
             tc.tile_pool(name="st", bufs=4) as stp, \
             tc.tile_pool(name="xr", bufs=3) as xrp, \
             tc.tile_pool(name="orp", bufs=2) as orp, \
             tc.tile_pool(name="ps", bufs=2, space="PSUM") as psp, \
             tc.tile_pool(name="ps2", bufs=2, space="PSUM") as psp2:
            bdw0 = wp.tile([128, 128], f32r, tag="bdw0")
            nc.sync.dma_start(bdw0[:], bdw0_d.ap())
            wx = wp.tile([G, 128], f32r, tag="wx")
            nc.sync.dma_start(wx[:], wx_d.ap())
            bdw1i = wp.tile([128, 128], f32r, tag="bdw1i")
            nc.sync.dma_start(bdw1i[:], bdw1i_d.ap())
            bdw1h = wp.tile([128, 128], f32r, tag="bdw1h")
            nc.sync.dma_start(bdw1h[:], bdw1h_d.ap())
            wp1 = wp.tile([128, G], f32r, tag="wp1")
            nc.sync.dma_start(wp1[:], wp1_d.ap())
            wph = wp.tile([G, G], f32r, tag="wph")
            nc.sync.dma_start(wph[:], wph_d.ap())
            b0t = wp.tile([128, 1], f32, tag="b0")
            nc.sync.dma_start(b0t[:], b0_d.ap())
            b1t = wp.tile([128, 1], f32, tag="b1")
            nc.sync.dma_start(b1t[:], b1_d.ap())
            bpt = wp.tile([G, 1], f32, tag="bp")
            nc.sync.dma_start(bpt[:], bp_d.ap())

            h0_prev = stp.tile([128, B], f32r, tag="h0")
            nc.sync.dma_start(h0_prev[:], h0i_d.ap())
            h1_prev = stp.tile([128, B], f32r, tag="h1")
            nc.sync.dma_start(h1_prev[:], h1i_d.ap())
            hp0 = wp.tile([G, B], f32r, tag="hpi")
            nc.sync.dma_start(hp0[:], hpi_d.ap())
            hp_prev = hp0[:]

            xslab = None
            oslab = None
            for t in range(S):
                if t % SS == 0:
                    xslab = xrp.tile([G, SS * B], f32r, tag="xs")
                    nc.sync.dma_start(xslab[:], xs_d.ap()[:, t * B:(t + SS) * B])
                    oslab = orp.tile([G, SS * B], f32r, tag="ys")
                k = t % SS
                xt = xslab[:, k * B:(k + 1) * B]

                ps0 = psp.tile([128, B], f32, tag="ps0")
                nc.tensor.matmul(ps0[:], wx[:], xt, start=True, stop=False)
                nc.tensor.matmul(ps0[:], bdw0[:], h0_prev[:], start=False, stop=True)
                h0_t = stp.tile([128, B], f32r, tag="h0")
                nc.scalar.activation(h0_t[:], ps0[:], Tanh, bias=b0t[:])

                ps1 = psp.tile([128, B], f32, tag="ps1")
                nc.tensor.matmul(ps1[:], bdw1i[:], h0_t[:], start=True, stop=False)
                nc.tensor.matmul(ps1[:], bdw1h[:], h1_prev[:], start=False, stop=True)
                h1_t = stp.tile([128, B], f32r, tag="h1")
                nc.scalar.activation(h1_t[:], ps1[:], Tanh, bias=b1t[:])

                ps2 = psp2.tile([G, B], f32, tag="ps2")
                nc.tensor.matmul(ps2[:], wp1[:], h1_t[:], start=True, stop=False)
                nc.tensor.matmul(ps2[:], wph[:], hp_prev, start=False, stop=True)
                hp_t = oslab[:, k * B:(k + 1) * B]
                nc.scalar.activation(hp_t, ps2[:], Tanh, bias=bpt[:])
                hp_prev = hp_t

                h0_prev, h1_prev = h0_t, h1_t
                if k == SS - 1:
                    nc.sync.dma_start(
                        ys_d.ap()[:, (t - SS + 1) * B:(t + 1) * B], oslab[:]
                    )
    nc.compile()
    return nc


def _get_program():
    if "nc" not in _cache:
        _cache["nc"] = _build_program()
    return _cache["nc"]


def kernel(x, x_lb, x_ub, W_ih0, W_hh0, b_ih0, b_hh0,
           W_ih1, W_hh1, b_ih1, b_hh1,
           Wp_ih, Wp_hh, bp_ih, bp_hh, prev_h0, post_h0):
    from concourse import bass_utils

    x = np.asarray(x, np.float32).reshape(T)
    lb = float(np.asarray(x_lb)); ub = float(np.asarray(x_ub))
    W_ih0 = np.asarray(W_ih0, np.float32); W_hh0 = np.asarray(W_hh0, np.float32)
    W_ih1 = np.asarray(W_ih1, np.float32); W_hh1 = np.asarray(W_hh1, np.float32)
    Wp_ih = np.asarray(Wp_ih, np.float32); Wp_hh = np.asarray(Wp_hh, np.float32)
    prev_h0 = np.asarray(prev_h0, np.float32)
    post_h0 = np.asarray(post_h0, np.float32)

    scale = 1.0 / (ub - lb)
    wih0 = W_ih0[:, 0]
    wih0_eff = wih0 * scale
    b0_eff = (np.asarray(b_ih0, np.float32) + np.asarray(b_hh0, np.float32)
              - wih0 * (lb * scale))
    b1_eff = np.asarray(b_ih1, np.float32) + np.asarray(b_hh1, np.float32)
    bp_eff = float(np.asarray(bp_ih, np.float32)[0] + np.asarray(bp_hh, np.float32)[0])

    # weight operands (lhsT layouts), f32r-rounded on host
    bdw0 = _round_f32r(_blockdiag4(W_hh0.T))
    wx = np.zeros((G, 128), np.float32)
    for g in range(G):
        wx[g, 32 * g:32 * g + 32] = wih0_eff
    wx = _round_f32r(wx)
    bdw1i = _round_f32r(_blockdiag4(W_ih1.T))
    bdw1h = _round_f32r(_blockdiag4(W_hh1.T))
    wp1 = np.zeros((128, G), np.float32)
    for g in range(G):
        wp1[32 * g:32 * g + 32, g] = Wp_ih[0]
    wp1 = _round_f32r(wp1)
    wph = _round_f32r(np.eye(G, dtype=np.float32) * float(Wp_hh[0, 0]))
    b0c = np.tile(b0_eff, G).reshape(128, 1).astype(np.float32)
    b1c = np.tile(b1_eff, G).reshape(128, 1).astype(np.float32)
    bpc = np.full((G, 1), bp_eff, np.float32)

    # per-core inputs
    xr = _round_f32r(x)
    in_maps = []
    for r in range(NCORES):
        # chunk c = r*CH + g*B + j ; window start w_c = c*L - TAU (c>=1), 0 for c=0
        cs = r * CH + (np.arange(G)[:, None] * B + np.arange(B)[None, :])  # [G,B]
        w = cs * L - TAU
        idx = w[:, :, None] + np.arange(S)[None, None, :]          # [G,B,S]
        if r == 0:
            idx[0, 0, :] = np.arange(S)                            # chunk 0: true window
        np.clip(idx, 0, T - 1, out=idx)
        xs = xr[idx].transpose(0, 2, 1).reshape(G, S * B)          # [g, t*B+j]

        h0i = np.zeros((128, B), np.float32)
        h1i = np.zeros((128, B), np.float32)
        hpi = np.zeros((G, B), np.float32)
        if r == 0:
            h0i[0:32, 0] = _round_f32r(prev_h0[0])
            h1i[0:32, 0] = _round_f32r(prev_h0[1])
            hpi[0, 0] = float(_round_f32r(np.float32(post_h0[0, 0])).reshape(()))
        in_maps.append({
            "xs": xs, "h0i": h0i, "h1i": h1i, "hpi": hpi,
            "bdw0": bdw0, "wx": wx, "bdw1i": bdw1i, "bdw1h": bdw1h,
            "wp1": wp1, "wph": wph, "b0": b0c, "b1": b1c, "bp": bpc,
        })

    nc = _get_program()
    _cache["last_in_maps"] = in_maps
    res = bass_utils.run_bass_kernel_spmd(nc, in_maps, core_ids=list(range(NCORES)))

    y = np.empty(T, np.float32)
    for r in range(NCORES):
        O = res.results[r]["ys"].reshape(G, S, B)
        yc = O[:, TAU:S, :].transpose(0, 2, 1).reshape(-1)  # (g, j, t) order
        y[r * CH * L:(r + 1) * CH * L] = yc
    O0 = res.results[0]["ys"].reshape(G, S, B)
    y[0:L] = O0[0, 0:L, 0]
    return y.reshape(T, 1)
